# revision 8
# baseline (speedup 1.0000x reference)
"""Trainium2 Bass kernel for nn_ContextualViewModel (gnn_message_passing).

Reference semantics:
    sx, sy = station_ids // 512, station_ids % 512
    s = sum_k x[sx_k, sy_k] @ W          # a single (128,) vector
    out = broadcast_to(s, (512, 512, 128))

The compute is tiny; the problem is memory-bound on writing the 128 MiB
output. Sharding: split the (i,j) grid of the output across 8 cores
(64 rows of 512 each -> 16 MiB per core). The K=128 gathered station rows
and W are replicated to every core (gathered host-side while slicing
inputs, per the sharding hint).

Per-core pipeline (measured on HW via perfetto/NTFF):
  - One packed load [gT | W] -> SBUF (one DMA, one sem wait).
  - ACT row-sum (accum_out) gives u[c] = sum_k g[k,c] without a matmul.
  - One PE matmul with a 0-stride broadcast lhsT yields b[p,d] = s[d] on
    all 128 partitions.
  - DVE and ACT copy the two halves of a [128, 2048] replicated tile in
    parallel (both read PSUM with 0-stride repeat), so the full-width
    tile is ready as soon as the slower copy finishes.
  - The 16 MiB shard streams out as 20 plain [lines, 2048]-float stores
    on the two HWDGE queues (sync/scalar). All descriptors are 8 KiB.

Store schedule rebalance: SDMA engine 15 (serving SBUF partitions
92-95 and 124-127) measures ~20% slower per byte than engines 0-14
(known TRN2 erratum). A uniform 128-partition stream makes it an
8-9 us straggler. The schedule below gives its partitions 13 lines
instead of 16 and routes the deficit through the other engines, so all
16 SDMA engines finish together. If engine 15 is NOT slow on a given
part, the cost of the skew is <1 us; if it is, this saves ~8 us.
"""

import sys

import numpy as np

try:
    import concourse  # noqa: F401
except ImportError:  # pragma: no cover
    sys.path.insert(0, "/opt/trn_rl_repo")

H, WD, K = 512, 512, 128
N_CORES = 8
ROWS_PER_CORE = H // N_CORES           # 64 rows of the (i) axis per core
SHARD_FLOATS = ROWS_PER_CORE * WD * K  # 4,194,304 floats = 16 MiB

LINE_F = 2048                          # floats per line (8 KiB descriptors)
N_LINES = SHARD_FLOATS // LINE_F       # 2048 lines per core
HALF_W = LINE_F // 2

# SDMA engine 15 serves partitions 92-95 and 124-127 (port swizzle
# port = ((p>>2)&7)<<1 | (p>>6)). Fast partition ranges exclude them.
FAST_A = (0, 92)
FAST_B = (96, 124)

_NC = None


def _store_schedule():
    """Return a list of (line0, line1, part0, part1) stores covering
    all N_LINES lines, rebalanced away from SDMA engine 15.

    Per partition line counts: slow {92-95,124-127}: 13; most fast: 16;
    partitions 64-87: 17. Per engine: eng15 104 lines, engines 1,3,5,7,
    9,11: 132, rest: 128 (of 8 KiB each).
    """
    sched = []
    ln = 0
    # 13 uniform chunks over all 128 partitions
    for _ in range(13):
        sched.append((ln, ln + 128, 0, 128))
        ln += 128
    # 3 rounds over the 120 fast partitions only
    for _ in range(3):
        for p0, p1 in (FAST_A, FAST_B):
            n = p1 - p0
            sched.append((ln, ln + n, p0, p1))
            ln += n
    # 24 stray lines through partitions 64..87 (engines 1,3,5,7,9,11)
    sched.append((ln, ln + 24, 64, 88))
    ln += 24
    assert ln == N_LINES, ln
    return sched


def _build():
    """Raw bacc build: manual semaphores, no Tile scheduling overhead."""
    from contextlib import ExitStack

    import concourse.bass as bass
    import concourse.bacc as bacc
    import concourse.mybir as mybir

    f32 = mybir.dt.float32
    nc = bacc.Bacc(
        "TRN2", target_bir_lowering=False, debug=False, num_devices=N_CORES
    )

    gw_dram = nc.dram_tensor("gw", [K, 2 * K], f32, kind="ExternalInput")
    out_dram = nc.dram_tensor("out", [N_LINES, LINE_F], f32, kind="ExternalOutput")

    sched = _store_schedule()
    n_stores = len(sched)
    sync_stores = sched[0::2]
    scalar_stores = sched[1::2]

    with ExitStack() as ctx:
        ec = ctx.enter_context
        gwt = ec(nc.sbuf_tensor("gwt", [K, 2 * K], f32))
        junk = ec(nc.sbuf_tensor("junk", [K, K], f32))
        r_sb = ec(nc.sbuf_tensor("r_sb", [K, 1], f32))
        rep = ec(nc.sbuf_tensor("rep", [128, LINE_F], f32))
        # Two PSUM banks with identical contents: DVE reads one while ACT
        # reads the other. Concurrent DVE+ACT reads of the SAME PSUM bank
        # hard-fault the device (measured: NRT_EXEC_UNIT_UNRECOVERABLE).
        b_ps = ec(nc.psum_tensor("b_ps", [128, K], f32))
        b_ps2 = ec(nc.psum_tensor("b_ps2", [128, K], f32))
        sem_in = ec(nc.semaphore("sem_in"))
        sem_r = ec(nc.semaphore("sem_r"))
        sem_p = ec(nc.semaphore("sem_p"))
        sem_v = ec(nc.semaphore("sem_v"))
        sem_a = ec(nc.semaphore("sem_a"))
        sem_out = ec(nc.semaphore("sem_out"))
        block = ec(nc.Block())

        # 0-stride repeat reads of PSUM: [128, K] viewed as [128, HALF_W]
        def _rep_view(ps):
            base = ps[:]
            return bass.AP(
                tensor=base.tensor,
                offset=base.offset,
                ap=[[K, 128], [0, HALF_W // K], [1, K]],
            )

        b_rep = _rep_view(b_ps)
        b_rep2 = _rep_view(b_ps2)

        @block.sync
        def _(sync):
            sync.dma_start(gwt[:], gw_dram[:]).then_inc(sem_in, 16)
            sync.wait_ge(sem_v, 1)
            sync.wait_ge(sem_a, 1)
            for ln0, ln1, p0, p1 in sync_stores:
                sync.dma_start(
                    out_dram[ln0:ln1], rep[p0:p1, :]
                ).then_inc(sem_out, 16)
            sync.wait_ge(sem_out, 16 * n_stores)

        @block.scalar
        def _(scalar):
            scalar.wait_ge(sem_in, 16)
            # u[c] = sum_k g[k, c]: row-sum of gT via the activation
            # accumulator (free-dim sum per partition)
            scalar.activation(
                junk[:],
                gwt[:, 0:K],
                mybir.ActivationFunctionType.Copy,
                accum_out=r_sb[:],
            ).then_inc(sem_r, 1)
            scalar.wait_ge(sem_p, 2)
            scalar.copy(rep[:, HALF_W:LINE_F], b_rep2).then_inc(sem_a, 1)
            scalar.wait_ge(sem_v, 1)
            scalar.wait_ge(sem_a, 1)
            for ln0, ln1, p0, p1 in scalar_stores:
                scalar.dma_start(
                    out_dram[ln0:ln1], rep[p0:p1, :]
                ).then_inc(sem_out, 16)

        @block.tensor
        def _(tensor):
            tensor.wait_ge(sem_r, 1)
            # lhsT[c, p] = u[c] via 0-stride broadcast ->
            # b[p, d] = sum_c u[c] W[c, d] = s[d] on every partition
            r_base = r_sb[:]
            r_bc = bass.AP(
                tensor=r_base.tensor, offset=r_base.offset, ap=[[1, K], [0, K]]
            )
            tensor.matmul(
                b_ps[:], r_bc, gwt[:, K : 2 * K], start=True, stop=True
            ).then_inc(sem_p, 1)
            tensor.matmul(
                b_ps2[:], r_bc, gwt[:, K : 2 * K], start=True, stop=True
            ).then_inc(sem_p, 1)

        @block.vector
        def _(vector):
            vector.wait_ge(sem_p, 1)
            vector.tensor_copy(rep[:, 0:HALF_W], b_rep).then_inc(sem_v, 1)

    nc.compile()
    return nc


def _get_nc():
    global _NC
    if _NC is None:
        _NC = _build()
    return _NC


def _prep_inputs(x: np.ndarray, W: np.ndarray, station_ids: np.ndarray):
    x = np.asarray(x, dtype=np.float32)
    W = np.asarray(W, dtype=np.float32)
    sid = np.asarray(station_ids).astype(np.int64)
    sx = sid // H
    sy = sid % WD
    g = x[sx, sy]  # (K, K) gathered station rows
    gw = np.ascontiguousarray(
        np.concatenate([g.T, W], axis=1), dtype=np.float32
    )  # [K, 2K]: columns 0:K = g^T, K:2K = W
    return gw


def _run(gw: np.ndarray, trace: bool = False):
    from concourse.bass_utils import run_bass_kernel_spmd

    nc = _get_nc()
    in_maps = [{"gw": gw} for _ in range(N_CORES)]
    return run_bass_kernel_spmd(nc, in_maps, list(range(N_CORES)), trace=trace)


def kernel(x: np.ndarray, W: np.ndarray, station_ids: np.ndarray) -> np.ndarray:
    gw = _prep_inputs(x, W, station_ids)
    res = _run(gw).results
    shards = [res[c]["out"].reshape(ROWS_PER_CORE, WD, K) for c in range(N_CORES)]
    return np.concatenate(shards, axis=0)


# revision 9
# speedup vs baseline: 1.2212x; 1.2212x over previous
"""Trainium2 Bass kernel for nn_ContextualViewModel (gnn_message_passing).

Reference semantics:
    sx, sy = station_ids // 512, station_ids % 512
    s = sum_k x[sx_k, sy_k] @ W          # a single (128,) vector
    out = broadcast_to(s, (512, 512, 128))

The compute is tiny; the problem is memory-bound on writing the 128 MiB
output. Sharding: split the (i,j) grid of the output across 8 cores
(64 rows of 512 each -> 16 MiB per core). The K=128 gathered station rows
and W are replicated to every core (gathered host-side while slicing
inputs, per the sharding hint).

Per-core pipeline (measured on HW via perfetto/NTFF):
  - One packed load [gT | W] -> SBUF (one DMA, one sem wait).
  - ACT row-sum (accum_out) gives u[c] = sum_k g[k,c] without a matmul.
  - One PE matmul with a 0-stride broadcast lhsT yields b[p,d] = s[d] on
    all 128 partitions.
  - DVE and ACT copy the two halves of a [128, 2048] replicated tile in
    parallel (both read PSUM with 0-stride repeat), so the full-width
    tile is ready as soon as the slower copy finishes.
  - The 16 MiB shard streams out as 20 plain [lines, 2048]-float stores
    on the two HWDGE queues (sync/scalar). All descriptors are 8 KiB.

Store schedule rebalance: SDMA engine 15 (serving SBUF partitions
92-95 and 124-127) measures ~20% slower per byte than engines 0-14
(known TRN2 erratum). A uniform 128-partition stream makes it an
8-9 us straggler. The schedule below gives its partitions 13 lines
instead of 16 and routes the deficit through the other engines, so all
16 SDMA engines finish together. If engine 15 is NOT slow on a given
part, the cost of the skew is <1 us; if it is, this saves ~8 us.
"""

import sys

import numpy as np

try:
    import concourse  # noqa: F401
except ImportError:  # pragma: no cover
    sys.path.insert(0, "/opt/trn_rl_repo")

H, WD, K = 512, 512, 128
N_CORES = 8
ROWS_PER_CORE = H // N_CORES           # 64 rows of the (i) axis per core
SHARD_FLOATS = ROWS_PER_CORE * WD * K  # 4,194,304 floats = 16 MiB

LINE_F = 2048                          # floats per line (8 KiB descriptors)
N_LINES = SHARD_FLOATS // LINE_F       # 2048 lines per core
HALF_W = LINE_F // 2

# SDMA engine 15 serves partitions 92-95 and 124-127 (port swizzle
# port = ((p>>2)&7)<<1 | (p>>6)). Fast partition ranges exclude them.
FAST_A = (0, 92)
FAST_B = (96, 124)

_NC = None


def _store_schedule():
    """Return a list of (line0, line1, part0, part1) stores covering
    all N_LINES lines, rebalanced away from SDMA engine 15.

    HWDGE descriptor spread (reverse-engineered from NTFF traces): an
    n-line store is split over m = n/d engines (d = smallest divisor of
    n with d >= n/16), filling engine slots from 0. The source
    partition range is irrelevant to the engine choice. So:
      128 lines -> 16 engines x 8      120 lines -> engines 0-14 x 8
       24 lines -> engines 0-11 x 2
    Schedule: 13x128 + 3x120 + 1x24 gives engine 15 104 lines, engines
    12-14 128, engines 0-11 130 — about the 0.81x skew that matches
    engine 15's measured slow-packet rate (309 -> 384 ns/8 KiB).
    """
    sched = []
    ln = 0
    # 13 uniform chunks over all 128 partitions (16 engines x 8 each)
    for _ in range(13):
        sched.append((ln, ln + 128, 0, 128))
        ln += 128
    # 3 stores of 120 lines (engines 0-14 x 8 each; engine 15 spared)
    for _ in range(3):
        sched.append((ln, ln + 120, 0, 120))
        ln += 120
    # 24 stray lines (engines 0-11 x 2)
    sched.append((ln, ln + 24, 0, 24))
    ln += 24
    assert ln == N_LINES, ln
    return sched


def _build():
    """Raw bacc build: manual semaphores, no Tile scheduling overhead."""
    from contextlib import ExitStack

    import concourse.bass as bass
    import concourse.bacc as bacc
    import concourse.mybir as mybir

    f32 = mybir.dt.float32
    nc = bacc.Bacc(
        "TRN2", target_bir_lowering=False, debug=False, num_devices=N_CORES
    )

    gw_dram = nc.dram_tensor("gw", [K, 2 * K], f32, kind="ExternalInput")
    out_dram = nc.dram_tensor("out", [N_LINES, LINE_F], f32, kind="ExternalOutput")

    sched = _store_schedule()
    n_stores = len(sched)
    sync_stores = sched[0::2]
    scalar_stores = sched[1::2]

    with ExitStack() as ctx:
        ec = ctx.enter_context
        gwt = ec(nc.sbuf_tensor("gwt", [K, 2 * K], f32))
        junk = ec(nc.sbuf_tensor("junk", [K, K], f32))
        r_sb = ec(nc.sbuf_tensor("r_sb", [K, 1], f32))
        rep = ec(nc.sbuf_tensor("rep", [128, LINE_F], f32))
        # Two PSUM banks with identical contents: DVE reads one while ACT
        # reads the other. Concurrent DVE+ACT reads of the SAME PSUM bank
        # hard-fault the device (measured: NRT_EXEC_UNIT_UNRECOVERABLE).
        b_ps = ec(nc.psum_tensor("b_ps", [128, K], f32))
        b_ps2 = ec(nc.psum_tensor("b_ps2", [128, K], f32))
        sem_in = ec(nc.semaphore("sem_in"))
        sem_r = ec(nc.semaphore("sem_r"))
        sem_p = ec(nc.semaphore("sem_p"))
        sem_v = ec(nc.semaphore("sem_v"))
        sem_a = ec(nc.semaphore("sem_a"))
        sem_out = ec(nc.semaphore("sem_out"))
        block = ec(nc.Block())

        # 0-stride repeat reads of PSUM: [128, K] viewed as [128, HALF_W]
        def _rep_view(ps):
            base = ps[:]
            return bass.AP(
                tensor=base.tensor,
                offset=base.offset,
                ap=[[K, 128], [0, HALF_W // K], [1, K]],
            )

        b_rep = _rep_view(b_ps)
        b_rep2 = _rep_view(b_ps2)

        @block.sync
        def _(sync):
            sync.dma_start(gwt[:], gw_dram[:]).then_inc(sem_in, 16)
            sync.wait_ge(sem_v, 1)
            sync.wait_ge(sem_a, 1)
            for ln0, ln1, p0, p1 in sync_stores:
                sync.dma_start(
                    out_dram[ln0:ln1], rep[p0:p1, :]
                ).then_inc(sem_out, 16)
            sync.wait_ge(sem_out, 16 * n_stores)

        @block.scalar
        def _(scalar):
            scalar.wait_ge(sem_in, 16)
            # u[c] = sum_k g[k, c]: row-sum of gT via the activation
            # accumulator (free-dim sum per partition)
            scalar.activation(
                junk[:],
                gwt[:, 0:K],
                mybir.ActivationFunctionType.Copy,
                accum_out=r_sb[:],
            ).then_inc(sem_r, 1)
            scalar.wait_ge(sem_p, 2)
            scalar.copy(rep[:, HALF_W:LINE_F], b_rep2).then_inc(sem_a, 1)
            scalar.wait_ge(sem_v, 1)
            scalar.wait_ge(sem_a, 1)
            for ln0, ln1, p0, p1 in scalar_stores:
                scalar.dma_start(
                    out_dram[ln0:ln1], rep[p0:p1, :]
                ).then_inc(sem_out, 16)

        @block.tensor
        def _(tensor):
            tensor.wait_ge(sem_r, 1)
            # lhsT[c, p] = u[c] via 0-stride broadcast ->
            # b[p, d] = sum_c u[c] W[c, d] = s[d] on every partition
            r_base = r_sb[:]
            r_bc = bass.AP(
                tensor=r_base.tensor, offset=r_base.offset, ap=[[1, K], [0, K]]
            )
            tensor.matmul(
                b_ps[:], r_bc, gwt[:, K : 2 * K], start=True, stop=True
            ).then_inc(sem_p, 1)
            tensor.matmul(
                b_ps2[:], r_bc, gwt[:, K : 2 * K], start=True, stop=True
            ).then_inc(sem_p, 1)

        @block.vector
        def _(vector):
            vector.wait_ge(sem_p, 1)
            vector.tensor_copy(rep[:, 0:HALF_W], b_rep).then_inc(sem_v, 1)

    nc.compile()
    return nc


def _get_nc():
    global _NC
    if _NC is None:
        _NC = _build()
    return _NC


def _prep_inputs(x: np.ndarray, W: np.ndarray, station_ids: np.ndarray):
    x = np.asarray(x, dtype=np.float32)
    W = np.asarray(W, dtype=np.float32)
    sid = np.asarray(station_ids).astype(np.int64)
    sx = sid // H
    sy = sid % WD
    g = x[sx, sy]  # (K, K) gathered station rows
    gw = np.ascontiguousarray(
        np.concatenate([g.T, W], axis=1), dtype=np.float32
    )  # [K, 2K]: columns 0:K = g^T, K:2K = W
    return gw


def _run(gw: np.ndarray, trace: bool = False):
    from concourse.bass_utils import run_bass_kernel_spmd

    nc = _get_nc()
    in_maps = [{"gw": gw} for _ in range(N_CORES)]
    return run_bass_kernel_spmd(nc, in_maps, list(range(N_CORES)), trace=trace)


def kernel(x: np.ndarray, W: np.ndarray, station_ids: np.ndarray) -> np.ndarray:
    gw = _prep_inputs(x, W, station_ids)
    res = _run(gw).results
    shards = [res[c]["out"].reshape(ROWS_PER_CORE, WD, K) for c in range(N_CORES)]
    return np.concatenate(shards, axis=0)
